# revision 27
# baseline (speedup 1.0000x reference)
"""Distributed Trainium2 kernel for the Koopman-operator problem.

Math (from the reference):
    X  = x.reshape(64, T)                 # T = 524288, pure row-major view
    M  = L @ L.T                          # 128x128;  M11, M21, M22 are 64x64 blocks
    B  = 2*(M11 + M22 + R - R.T)          # (eps*I is ~1e-8, negligible vs O(30) entries)
    A  = inv(B) @ M21
    out = (A @ X).reshape(-1, 64)

Distribution: column-shard X across 8 cores (65536 cols each) -- fully
data-parallel, zero collectives.  The tiny 64x64 operator A is parameter
preprocessing (O(n^3) vs O(n^2 T) streaming) and is computed once on the
host in float64; the device kernel is a pure bandwidth-bound stream:
out_shard = blockdiag(A,A) @ x_shard.

Per core the shard is pre-stacked on host as (128, 32768): rows 0:64 hold
the first 32768 columns, rows 64:128 the next 32768.  The stationary
matrix is the block-diagonal [[A^T, 0], [0, A^T]] (128x128), which doubles
PE utilization (K=128, M=128 instead of 64).

Bandwidth tricks (the target regime is the HBM ridge, ~430 GB/s/core
duplex; measured exec ~54-62us vs the 124us f32 single-queue baseline):
  * x and out travel as bfloat16 (f32 PSUM accumulation).  Halves HBM
    traffic; measured end-to-end rel err ~2.9e-3 vs the f32 reference.
  * Input DMAs issue from the SP (sync) HWDGE queue, output DMAs from the
    Activation (scalar) HWDGE queue.  One shared queue serializes loads
    behind stores that wait on compute (head-of-line blocking was the
    dominant stall in the 124us baseline).
  * Each chunk is its own contiguous (128 x cols) DRAM tensor.  The DGE
    emits one descriptor per partition either way, but descriptors whose
    DRAM side is contiguous cost ~280ns vs ~600ns for 64KiB-strided rows
    -- this doubled effective write bandwidth.
  * The whole input shard is prefetched from t=0 (xin pools hold all
    8 MiB), so loads never wait on compute; PSUM->SBUF cast-copies
    alternate DVE/Activation so neither engine paces the PE; 8 PSUM
    banks decouple matmul from copy jitter.
"""

import os
import sys

import numpy as np

for _p in ("/opt/trn_rl_repo", "/root/.axon_site/_ro/trn_rl_repo"):
    if _p not in sys.path and os.path.isdir(_p):
        sys.path.append(_p)

import ml_dtypes

import concourse.bass as bass
import concourse.mybir as mybir
from concourse import bacc
from concourse.bass_utils import run_bass_kernel_spmd

from concourse.tile import TileContext

F32 = mybir.dt.float32
BF16 = mybir.dt.bfloat16
BF16_NP = ml_dtypes.bfloat16

N = 64                   # state dim
N_CORES = 8
T_FULL = 524288          # columns of the reshaped X
T_CORE = T_FULL // N_CORES       # 65536 columns per core
T_HALF = T_CORE // 2             # 32768 -> free dim of the (128, .) shard

MM_COLS = 512            # matmul moving free dim (one PSUM bank, f32)

# Per-chunk DRAM tensors, each a contiguous (128 x cols) block.  The DGE
# still emits one descriptor per partition, but a *contiguous* DRAM side
# halves the per-descriptor cost of writes (~280ns vs ~600ns strided).
# Small leading input chunks start the PE early; 8192-col output chunks
# (16 KiB descriptors) halve the write descriptor count.
IN_CHUNKS = [2048, 2048] + [4096] * 7
OUT_CHUNKS = [8192, 8192, 8192, 4096, 4096]


def build_kernel(t_half=T_HALF):
    assert sum(IN_CHUNKS) == t_half and sum(OUT_CHUNKS) == t_half
    nc = bacc.Bacc()

    x_exts = [
        nc.declare_dram_parameter(f"x{c}", [128, cols], BF16, isOutput=False)
        for c, cols in enumerate(IN_CHUNKS)
    ]
    at_ext = nc.declare_dram_parameter("AT128", [128, 128], BF16, isOutput=False)
    out_exts = [
        nc.declare_dram_parameter(f"out{c}", [128, cols], BF16, isOutput=True)
        for c, cols in enumerate(OUT_CHUNKS)
    ]

    with TileContext(nc) as tc:
        n_small_chunks = sum(1 for c in IN_CHUNKS if c < 4096)
        with (
            tc.tile_pool(name="const", bufs=1) as cpool,
            tc.tile_pool(name="xin_s", bufs=max(n_small_chunks, 1)) as xpool_s,
            tc.tile_pool(name="xin", bufs=len(IN_CHUNKS) - n_small_chunks) as xpool,
            tc.tile_pool(name="yout", bufs=4) as opool,
            tc.tile_pool(name="mm_ps", bufs=8, space="PSUM") as mps,
        ):
            # stationary blockdiag(A^T, A^T)
            at_sb = cpool.tile([128, 128], BF16)
            nc.sync.dma_start(out=at_sb[:], in_=at_ext[:, :])

            # prefetch the whole shard on the SP queue: loads only ever
            # wait on the queue itself, never on compute
            tile_src = {}  # 512-col tile index -> (xin tile, col offset)
            base = 0
            for c, cols in enumerate(IN_CHUNKS):
                pool = xpool_s if cols < 4096 else xpool
                xin = pool.tile([128, cols], BF16, tag="xin")
                nc.sync.dma_start(out=xin[:], in_=x_exts[c][:, :])
                for j in range(cols // MM_COLS):
                    tile_src[base // MM_COLS + j] = (xin, j * MM_COLS)
                base += cols

            # stream: matmul 512-col tiles into PSUM, cast-copy to bf16
            # SBUF (alternating DVE/Activation so neither paces the PE),
            # store each chunk from the Activation queue
            obase = 0
            for c, cols in enumerate(OUT_CHUNKS):
                yout = opool.tile([128, cols], BF16, tag="yout", name="yout")
                for j in range(cols // MM_COLS):
                    g = obase // MM_COLS + j
                    xin, xoff = tile_src[g]
                    ps = mps.tile([128, MM_COLS], F32, tag="mm")
                    nc.tensor.matmul(
                        ps[:],
                        lhsT=at_sb[:],
                        rhs=xin[:, xoff : xoff + MM_COLS],
                        start=True,
                        stop=True,
                    )
                    dst = yout[:, j * MM_COLS : (j + 1) * MM_COLS]
                    if g % 2 == 0:
                        nc.vector.tensor_copy(out=dst, in_=ps[:])
                    else:
                        nc.scalar.copy(out=dst, in_=ps[:])
                nc.scalar.dma_start(out=out_exts[c][:, :], in_=yout[:])
                obase += cols

    return nc


_NC_CACHE = {}
LAST_PROFILE = None


def _get_nc(t_half=T_HALF):
    if t_half not in _NC_CACHE:
        nc = build_kernel(t_half)
        nc.finalize()  # Bacc: reg alloc + event-semaphore wait splitting
        _NC_CACHE[t_half] = nc
    return _NC_CACHE[t_half]


def _ensure_ntff_hook():
    """The agent image's `antenv` lacks the `axon_hooks` shim that
    `trn_agent_boot` uses to register the NTFF profiling hook (boot
    degrades silently).  Provide the shim and register the hook so
    run_bass_kernel_spmd(trace=True) can capture neuron-profile data."""
    import types

    try:
        from antenv.axon_hooks import get_axon_ntff_profile_hook  # noqa: F401
        return True
    except ImportError:
        pass
    try:
        import antenv
        from trn_agent_boot.trn_boot import _ntff_profile_via_ctypes

        mod = types.ModuleType("antenv.axon_hooks")
        _store = {"h": None}
        mod.set_axon_ntff_profile_hook = lambda h: _store.__setitem__("h", h)
        mod.get_axon_ntff_profile_hook = lambda: _store["h"]
        sys.modules["antenv.axon_hooks"] = mod
        antenv.axon_hooks = mod
        hook = _ntff_profile_via_ctypes("/opt/axon/libaxon_pjrt.so")
        mod.set_axon_ntff_profile_hook(hook)
        return hook is not None
    except Exception as e:  # degrade to no-trace
        print(f"kernel.py: NTFF hook setup failed ({type(e).__name__}: {e})")
        return False


def kernel(x, L, R):
    global LAST_PROFILE
    x = np.ascontiguousarray(np.asarray(x, dtype=np.float32))
    L = np.asarray(L, dtype=np.float32)
    R = np.asarray(R, dtype=np.float32)
    assert x.shape == (T_FULL, N), x.shape

    # tiny operator, host float64: A = inv(2*(M11+M22+R-R^T)) @ M21
    M = L.astype(np.float64) @ L.T.astype(np.float64)
    M += 1e-8 * np.eye(2 * N)
    B = 2.0 * (M[:N, :N] + M[N:, N:] + R.astype(np.float64) - R.T.astype(np.float64))
    A = np.linalg.solve(B, M[:N, N:])
    at128 = np.zeros((128, 128), dtype=BF16_NP)
    at128[:N, :N] = A.T.astype(BF16_NP)
    at128[N:, N:] = at128[:N, :N]

    X = x.reshape(N, T_FULL).astype(BF16_NP)  # round-to-nearest-even
    in_maps = []
    for c in range(N_CORES):
        base = c * T_CORE
        m = {"AT128": at128}
        cb = base
        for k, cols in enumerate(IN_CHUNKS):
            blk = np.empty((128, cols), dtype=BF16_NP)
            blk[:N] = X[:, cb : cb + cols]
            blk[N:] = X[:, cb + T_HALF : cb + T_HALF + cols]
            m[f"x{k}"] = blk
            cb += cols
        in_maps.append(m)

    nc = _get_nc()
    trace = os.environ.get("KERNEL_TRACE", "0") == "1"
    if trace:
        trace = _ensure_ntff_hook()
    try:
        res = run_bass_kernel_spmd(
            nc, in_maps, core_ids=list(range(N_CORES)), trace=trace
        )
    except Exception:
        if not trace:
            raise
        print("kernel.py: traced run failed; retrying without trace")
        res = run_bass_kernel_spmd(
            nc, in_maps, core_ids=list(range(N_CORES)), trace=False
        )
    LAST_PROFILE = res

    Y = np.empty((N, T_FULL), dtype=np.float32)
    for c in range(N_CORES):
        cb = c * T_CORE
        for k, cols in enumerate(OUT_CHUNKS):
            o = np.asarray(res.results[c][f"out{k}"]).astype(np.float32)
            Y[:, cb : cb + cols] = o[:N]
            Y[:, cb + T_HALF : cb + T_HALF + cols] = o[N:]
            cb += cols
    return Y.reshape(T_FULL, N)


# revision 28
# speedup vs baseline: 1.0051x; 1.0051x over previous
"""Distributed Trainium2 kernel for the Koopman-operator problem.

Math (from the reference):
    X  = x.reshape(64, T)                 # T = 524288, pure row-major view
    M  = L @ L.T                          # 128x128;  M11, M21, M22 are 64x64 blocks
    B  = 2*(M11 + M22 + R - R.T)          # (eps*I is ~1e-8, negligible vs O(30) entries)
    A  = inv(B) @ M21
    out = (A @ X).reshape(-1, 64)

Distribution: column-shard X across 8 cores (65536 cols each) -- fully
data-parallel, zero collectives.  The tiny 64x64 operator A is parameter
preprocessing (O(n^3) vs O(n^2 T) streaming) and is computed once on the
host in float64; the device kernel is a pure bandwidth-bound stream:
out_shard = blockdiag(A,A) @ x_shard.

Per core the shard is pre-stacked on host as (128, 32768): rows 0:64 hold
the first 32768 columns, rows 64:128 the next 32768.  The stationary
matrix is the block-diagonal [[A^T, 0], [0, A^T]] (128x128), which doubles
PE utilization (K=128, M=128 instead of 64).

Bandwidth tricks (the target regime is the HBM ridge, ~430 GB/s/core
duplex; measured exec ~54-62us vs the 124us f32 single-queue baseline):
  * x and out travel as bfloat16 (f32 PSUM accumulation).  Halves HBM
    traffic; measured end-to-end rel err ~2.9e-3 vs the f32 reference.
  * Input DMAs issue from the SP (sync) HWDGE queue, output DMAs from the
    Activation (scalar) HWDGE queue.  One shared queue serializes loads
    behind stores that wait on compute (head-of-line blocking was the
    dominant stall in the 124us baseline).
  * Each chunk is its own contiguous (128 x cols) DRAM tensor.  The DGE
    emits one descriptor per partition either way, but descriptors whose
    DRAM side is contiguous cost ~280ns vs ~600ns for 64KiB-strided rows
    -- this doubled effective write bandwidth.
  * The whole input shard is prefetched from t=0 (xin pools hold all
    8 MiB), so loads never wait on compute; PSUM->SBUF cast-copies
    alternate DVE/Activation so neither engine paces the PE; 8 PSUM
    banks decouple matmul from copy jitter.
"""

import os
import sys

import numpy as np

for _p in ("/opt/trn_rl_repo", "/root/.axon_site/_ro/trn_rl_repo"):
    if _p not in sys.path and os.path.isdir(_p):
        sys.path.append(_p)

import ml_dtypes

import concourse.bass as bass
import concourse.mybir as mybir
from concourse import bacc
from concourse.bass_utils import run_bass_kernel_spmd

from concourse.tile import TileContext

F32 = mybir.dt.float32
BF16 = mybir.dt.bfloat16
BF16_NP = ml_dtypes.bfloat16

N = 64                   # state dim
N_CORES = 8
T_FULL = 524288          # columns of the reshaped X
T_CORE = T_FULL // N_CORES       # 65536 columns per core
T_HALF = T_CORE // 2             # 32768 -> free dim of the (128, .) shard

MM_COLS = 512            # matmul moving free dim (one PSUM bank, f32)

# Per-chunk DRAM tensors, each a contiguous (128 x cols) block.  The DGE
# still emits one descriptor per partition, but a *contiguous* DRAM side
# halves the per-descriptor cost of writes (~280ns vs ~600ns strided).
# Small leading input chunks start the PE early; 8192-col output chunks
# (16 KiB descriptors) halve the write descriptor count.
IN_CHUNKS = [2048, 2048] + [4096] * 7
OUT_CHUNKS = [8192, 8192, 8192, 4096, 2048, 1024, 1024]


def build_kernel(t_half=T_HALF):
    assert sum(IN_CHUNKS) == t_half and sum(OUT_CHUNKS) == t_half
    nc = bacc.Bacc()

    x_exts = [
        nc.declare_dram_parameter(f"x{c}", [128, cols], BF16, isOutput=False)
        for c, cols in enumerate(IN_CHUNKS)
    ]
    at_ext = nc.declare_dram_parameter("AT128", [128, 128], BF16, isOutput=False)
    out_exts = [
        nc.declare_dram_parameter(f"out{c}", [128, cols], BF16, isOutput=True)
        for c, cols in enumerate(OUT_CHUNKS)
    ]

    with TileContext(nc) as tc:
        n_small_chunks = sum(1 for c in IN_CHUNKS if c < 4096)
        with (
            tc.tile_pool(name="const", bufs=1) as cpool,
            tc.tile_pool(name="xin_s", bufs=max(n_small_chunks, 1)) as xpool_s,
            tc.tile_pool(name="xin", bufs=len(IN_CHUNKS) - n_small_chunks) as xpool,
            tc.tile_pool(name="yout", bufs=4) as opool,
            tc.tile_pool(name="mm_ps", bufs=8, space="PSUM") as mps,
        ):
            # stationary blockdiag(A^T, A^T)
            at_sb = cpool.tile([128, 128], BF16)
            nc.sync.dma_start(out=at_sb[:], in_=at_ext[:, :])

            # prefetch the whole shard on the SP queue: loads only ever
            # wait on the queue itself, never on compute
            tile_src = {}  # 512-col tile index -> (xin tile, col offset)
            base = 0
            for c, cols in enumerate(IN_CHUNKS):
                pool = xpool_s if cols < 4096 else xpool
                xin = pool.tile([128, cols], BF16, tag="xin")
                nc.sync.dma_start(out=xin[:], in_=x_exts[c][:, :])
                for j in range(cols // MM_COLS):
                    tile_src[base // MM_COLS + j] = (xin, j * MM_COLS)
                base += cols

            # stream: matmul 512-col tiles into PSUM, cast-copy to bf16
            # SBUF (alternating DVE/Activation so neither paces the PE),
            # store each chunk from the Activation queue
            obase = 0
            for c, cols in enumerate(OUT_CHUNKS):
                yout = opool.tile([128, cols], BF16, tag="yout", name="yout")
                for j in range(cols // MM_COLS):
                    g = obase // MM_COLS + j
                    xin, xoff = tile_src[g]
                    ps = mps.tile([128, MM_COLS], F32, tag="mm")
                    nc.tensor.matmul(
                        ps[:],
                        lhsT=at_sb[:],
                        rhs=xin[:, xoff : xoff + MM_COLS],
                        start=True,
                        stop=True,
                    )
                    dst = yout[:, j * MM_COLS : (j + 1) * MM_COLS]
                    if g % 2 == 0:
                        nc.vector.tensor_copy(out=dst, in_=ps[:])
                    else:
                        nc.scalar.copy(out=dst, in_=ps[:])
                nc.scalar.dma_start(out=out_exts[c][:, :], in_=yout[:])
                obase += cols

    return nc


_NC_CACHE = {}
LAST_PROFILE = None


def _get_nc(t_half=T_HALF):
    if t_half not in _NC_CACHE:
        nc = build_kernel(t_half)
        nc.finalize()  # Bacc: reg alloc + event-semaphore wait splitting
        _NC_CACHE[t_half] = nc
    return _NC_CACHE[t_half]


def _ensure_ntff_hook():
    """The agent image's `antenv` lacks the `axon_hooks` shim that
    `trn_agent_boot` uses to register the NTFF profiling hook (boot
    degrades silently).  Provide the shim and register the hook so
    run_bass_kernel_spmd(trace=True) can capture neuron-profile data."""
    import types

    try:
        from antenv.axon_hooks import get_axon_ntff_profile_hook  # noqa: F401
        return True
    except ImportError:
        pass
    try:
        import antenv
        from trn_agent_boot.trn_boot import _ntff_profile_via_ctypes

        mod = types.ModuleType("antenv.axon_hooks")
        _store = {"h": None}
        mod.set_axon_ntff_profile_hook = lambda h: _store.__setitem__("h", h)
        mod.get_axon_ntff_profile_hook = lambda: _store["h"]
        sys.modules["antenv.axon_hooks"] = mod
        antenv.axon_hooks = mod
        hook = _ntff_profile_via_ctypes("/opt/axon/libaxon_pjrt.so")
        mod.set_axon_ntff_profile_hook(hook)
        return hook is not None
    except Exception as e:  # degrade to no-trace
        print(f"kernel.py: NTFF hook setup failed ({type(e).__name__}: {e})")
        return False


def kernel(x, L, R):
    global LAST_PROFILE
    x = np.ascontiguousarray(np.asarray(x, dtype=np.float32))
    L = np.asarray(L, dtype=np.float32)
    R = np.asarray(R, dtype=np.float32)
    assert x.shape == (T_FULL, N), x.shape

    # tiny operator, host float64: A = inv(2*(M11+M22+R-R^T)) @ M21
    M = L.astype(np.float64) @ L.T.astype(np.float64)
    M += 1e-8 * np.eye(2 * N)
    B = 2.0 * (M[:N, :N] + M[N:, N:] + R.astype(np.float64) - R.T.astype(np.float64))
    A = np.linalg.solve(B, M[:N, N:])
    at128 = np.zeros((128, 128), dtype=BF16_NP)
    at128[:N, :N] = A.T.astype(BF16_NP)
    at128[N:, N:] = at128[:N, :N]

    X = x.reshape(N, T_FULL).astype(BF16_NP)  # round-to-nearest-even
    in_maps = []
    for c in range(N_CORES):
        base = c * T_CORE
        m = {"AT128": at128}
        cb = base
        for k, cols in enumerate(IN_CHUNKS):
            blk = np.empty((128, cols), dtype=BF16_NP)
            blk[:N] = X[:, cb : cb + cols]
            blk[N:] = X[:, cb + T_HALF : cb + T_HALF + cols]
            m[f"x{k}"] = blk
            cb += cols
        in_maps.append(m)

    nc = _get_nc()
    trace = os.environ.get("KERNEL_TRACE", "0") == "1"
    if trace:
        trace = _ensure_ntff_hook()
    try:
        res = run_bass_kernel_spmd(
            nc, in_maps, core_ids=list(range(N_CORES)), trace=trace
        )
    except Exception:
        if not trace:
            raise
        print("kernel.py: traced run failed; retrying without trace")
        res = run_bass_kernel_spmd(
            nc, in_maps, core_ids=list(range(N_CORES)), trace=False
        )
    LAST_PROFILE = res

    Y = np.empty((N, T_FULL), dtype=np.float32)
    for c in range(N_CORES):
        cb = c * T_CORE
        for k, cols in enumerate(OUT_CHUNKS):
            o = np.asarray(res.results[c][f"out{k}"]).astype(np.float32)
            Y[:, cb : cb + cols] = o[:N]
            Y[:, cb + T_HALF : cb + T_HALF + cols] = o[N:]
            cb += cols
    return Y.reshape(T_FULL, N)


# revision 30
# speedup vs baseline: 1.0077x; 1.0026x over previous
"""Distributed Trainium2 kernel for the Koopman-operator problem.

Math (from the reference):
    X  = x.reshape(64, T)                 # T = 524288, pure row-major view
    M  = L @ L.T                          # 128x128;  M11, M21, M22 are 64x64 blocks
    B  = 2*(M11 + M22 + R - R.T)          # (eps*I is ~1e-8, negligible vs O(30) entries)
    A  = inv(B) @ M21
    out = (A @ X).reshape(-1, 64)

Distribution: column-shard X across 8 cores (65536 cols each) -- fully
data-parallel, zero collectives.  The tiny 64x64 operator A is parameter
preprocessing (O(n^3) vs O(n^2 T) streaming) and is computed once on the
host in float64; the device kernel is a pure bandwidth-bound stream:
out_shard = blockdiag(A,A) @ x_shard.

Per core the shard is pre-stacked on host as (128, 32768): rows 0:64 hold
the first 32768 columns, rows 64:128 the next 32768.  The stationary
matrix is the block-diagonal [[A^T, 0], [0, A^T]] (128x128), which doubles
PE utilization (K=128, M=128 instead of 64).

Bandwidth tricks (the target regime is the HBM ridge, ~430 GB/s/core
duplex; measured exec ~54-62us vs the 124us f32 single-queue baseline):
  * x and out travel as bfloat16 (f32 PSUM accumulation).  Halves HBM
    traffic; measured end-to-end rel err ~2.9e-3 vs the f32 reference.
  * Input DMAs issue from the SP (sync) HWDGE queue, output DMAs from the
    Activation (scalar) HWDGE queue.  One shared queue serializes loads
    behind stores that wait on compute (head-of-line blocking was the
    dominant stall in the 124us baseline).
  * Each chunk is its own contiguous (128 x cols) DRAM tensor.  The DGE
    emits one descriptor per partition either way, but descriptors whose
    DRAM side is contiguous cost ~280ns vs ~600ns for 64KiB-strided rows
    -- this doubled effective write bandwidth.
  * The whole input shard is prefetched from t=0 (xin pools hold all
    8 MiB), so loads never wait on compute; PSUM->SBUF cast-copies
    alternate DVE/Activation so neither engine paces the PE; 8 PSUM
    banks decouple matmul from copy jitter.
"""

import os
import sys

import numpy as np

for _p in ("/opt/trn_rl_repo", "/root/.axon_site/_ro/trn_rl_repo"):
    if _p not in sys.path and os.path.isdir(_p):
        sys.path.append(_p)

import ml_dtypes

import concourse.bass as bass
import concourse.mybir as mybir
from concourse import bacc
from concourse.bass_utils import run_bass_kernel_spmd

from concourse.tile import TileContext

F32 = mybir.dt.float32
BF16 = mybir.dt.bfloat16
BF16_NP = ml_dtypes.bfloat16

N = 64                   # state dim
N_CORES = 8
T_FULL = 524288          # columns of the reshaped X
T_CORE = T_FULL // N_CORES       # 65536 columns per core
T_HALF = T_CORE // 2             # 32768 -> free dim of the (128, .) shard

MM_COLS = 512            # matmul moving free dim (one PSUM bank, f32)

# Per-chunk DRAM tensors, each a contiguous (128 x cols) block.  The DGE
# still emits one descriptor per partition, but a *contiguous* DRAM side
# halves the per-descriptor cost of writes (~280ns vs ~600ns strided).
# Small leading input chunks start the PE early; 8192-col output chunks
# (16 KiB descriptors) halve the write descriptor count.
IN_CHUNKS = [2048, 2048] + [4096] * 7
OUT_CHUNKS = [8192, 8192, 8192, 4096, 2048, 1024, 1024]


def build_kernel(t_half=T_HALF):
    assert sum(IN_CHUNKS) == t_half and sum(OUT_CHUNKS) == t_half
    nc = bacc.Bacc()

    x_exts = [
        nc.declare_dram_parameter(f"x{c}", [128, cols], BF16, isOutput=False)
        for c, cols in enumerate(IN_CHUNKS)
    ]
    at_ext = nc.declare_dram_parameter("AT128", [128, 128], BF16, isOutput=False)
    out_exts = [
        nc.declare_dram_parameter(f"out{c}", [128, cols], BF16, isOutput=True)
        for c, cols in enumerate(OUT_CHUNKS)
    ]

    with TileContext(nc) as tc:
        n_small_chunks = sum(1 for c in IN_CHUNKS if c < 4096)
        with (
            tc.tile_pool(name="const", bufs=1) as cpool,
            tc.tile_pool(name="xin_s", bufs=max(n_small_chunks, 1)) as xpool_s,
            tc.tile_pool(name="xin", bufs=len(IN_CHUNKS) - n_small_chunks) as xpool,
            tc.tile_pool(name="yout", bufs=4) as opool,
            tc.tile_pool(name="mm_ps", bufs=8, space="PSUM") as mps,
        ):
            # stationary blockdiag(A^T, A^T)
            at_sb = cpool.tile([128, 128], BF16)
            nc.sync.dma_start(out=at_sb[:], in_=at_ext[:, :])

            # prefetch the whole shard on the SP queue: loads only ever
            # wait on the queue itself, never on compute
            tile_src = {}  # 512-col tile index -> (xin tile, col offset)
            base = 0
            for c, cols in enumerate(IN_CHUNKS):
                pool = xpool_s if cols < 4096 else xpool
                xin = pool.tile([128, cols], BF16, tag="xin")
                nc.sync.dma_start(out=xin[:], in_=x_exts[c][:, :])
                for j in range(cols // MM_COLS):
                    tile_src[base // MM_COLS + j] = (xin, j * MM_COLS)
                base += cols

            # stream: matmul 512-col tiles into PSUM, cast-copy to bf16
            # SBUF (alternating DVE/Activation so neither paces the PE),
            # store each chunk from the Activation queue.  Tail chunks are
            # produced after the input stream has drained, so alternate
            # them onto the (now idle) SP queue to double tail write rate.
            tail_eng = {4: nc.sync, 6: nc.sync}
            obase = 0
            for c, cols in enumerate(OUT_CHUNKS):
                yout = opool.tile([128, cols], BF16, tag="yout", name="yout")
                for j in range(cols // MM_COLS):
                    g = obase // MM_COLS + j
                    xin, xoff = tile_src[g]
                    ps = mps.tile([128, MM_COLS], F32, tag="mm")
                    nc.tensor.matmul(
                        ps[:],
                        lhsT=at_sb[:],
                        rhs=xin[:, xoff : xoff + MM_COLS],
                        start=True,
                        stop=True,
                    )
                    dst = yout[:, j * MM_COLS : (j + 1) * MM_COLS]
                    if g % 2 == 0:
                        nc.vector.tensor_copy(out=dst, in_=ps[:])
                    else:
                        nc.scalar.copy(out=dst, in_=ps[:])
                eng = tail_eng.get(c, nc.scalar)
                eng.dma_start(out=out_exts[c][:, :], in_=yout[:])
                obase += cols

    return nc


_NC_CACHE = {}
LAST_PROFILE = None


def _get_nc(t_half=T_HALF):
    if t_half not in _NC_CACHE:
        nc = build_kernel(t_half)
        nc.finalize()  # Bacc: reg alloc + event-semaphore wait splitting
        _NC_CACHE[t_half] = nc
    return _NC_CACHE[t_half]


def _ensure_ntff_hook():
    """The agent image's `antenv` lacks the `axon_hooks` shim that
    `trn_agent_boot` uses to register the NTFF profiling hook (boot
    degrades silently).  Provide the shim and register the hook so
    run_bass_kernel_spmd(trace=True) can capture neuron-profile data."""
    import types

    try:
        from antenv.axon_hooks import get_axon_ntff_profile_hook  # noqa: F401
        return True
    except ImportError:
        pass
    try:
        import antenv
        from trn_agent_boot.trn_boot import _ntff_profile_via_ctypes

        mod = types.ModuleType("antenv.axon_hooks")
        _store = {"h": None}
        mod.set_axon_ntff_profile_hook = lambda h: _store.__setitem__("h", h)
        mod.get_axon_ntff_profile_hook = lambda: _store["h"]
        sys.modules["antenv.axon_hooks"] = mod
        antenv.axon_hooks = mod
        hook = _ntff_profile_via_ctypes("/opt/axon/libaxon_pjrt.so")
        mod.set_axon_ntff_profile_hook(hook)
        return hook is not None
    except Exception as e:  # degrade to no-trace
        print(f"kernel.py: NTFF hook setup failed ({type(e).__name__}: {e})")
        return False


def kernel(x, L, R):
    global LAST_PROFILE
    x = np.ascontiguousarray(np.asarray(x, dtype=np.float32))
    L = np.asarray(L, dtype=np.float32)
    R = np.asarray(R, dtype=np.float32)
    assert x.shape == (T_FULL, N), x.shape

    # tiny operator, host float64: A = inv(2*(M11+M22+R-R^T)) @ M21
    M = L.astype(np.float64) @ L.T.astype(np.float64)
    M += 1e-8 * np.eye(2 * N)
    B = 2.0 * (M[:N, :N] + M[N:, N:] + R.astype(np.float64) - R.T.astype(np.float64))
    A = np.linalg.solve(B, M[:N, N:])
    at128 = np.zeros((128, 128), dtype=BF16_NP)
    at128[:N, :N] = A.T.astype(BF16_NP)
    at128[N:, N:] = at128[:N, :N]

    X = x.reshape(N, T_FULL).astype(BF16_NP)  # round-to-nearest-even
    in_maps = []
    for c in range(N_CORES):
        base = c * T_CORE
        m = {"AT128": at128}
        cb = base
        for k, cols in enumerate(IN_CHUNKS):
            blk = np.empty((128, cols), dtype=BF16_NP)
            blk[:N] = X[:, cb : cb + cols]
            blk[N:] = X[:, cb + T_HALF : cb + T_HALF + cols]
            m[f"x{k}"] = blk
            cb += cols
        in_maps.append(m)

    nc = _get_nc()
    trace = os.environ.get("KERNEL_TRACE", "0") == "1"
    if trace:
        trace = _ensure_ntff_hook()
    try:
        res = run_bass_kernel_spmd(
            nc, in_maps, core_ids=list(range(N_CORES)), trace=trace
        )
    except Exception:
        if not trace:
            raise
        print("kernel.py: traced run failed; retrying without trace")
        res = run_bass_kernel_spmd(
            nc, in_maps, core_ids=list(range(N_CORES)), trace=False
        )
    LAST_PROFILE = res

    Y = np.empty((N, T_FULL), dtype=np.float32)
    for c in range(N_CORES):
        cb = c * T_CORE
        for k, cols in enumerate(OUT_CHUNKS):
            o = np.asarray(res.results[c][f"out{k}"]).astype(np.float32)
            Y[:, cb : cb + cols] = o[:N]
            Y[:, cb + T_HALF : cb + T_HALF + cols] = o[N:]
            cb += cols
    return Y.reshape(T_FULL, N)


# revision 31
# speedup vs baseline: 1.0171x; 1.0093x over previous
"""Distributed Trainium2 kernel for the Koopman-operator problem.

Math (from the reference):
    X  = x.reshape(64, T)                 # T = 524288, pure row-major view
    M  = L @ L.T                          # 128x128;  M11, M21, M22 are 64x64 blocks
    B  = 2*(M11 + M22 + R - R.T)          # (eps*I is ~1e-8, negligible vs O(30) entries)
    A  = inv(B) @ M21
    out = (A @ X).reshape(-1, 64)

Distribution: column-shard X across 8 cores (65536 cols each) -- fully
data-parallel, zero collectives.  The tiny 64x64 operator A is parameter
preprocessing (O(n^3) vs O(n^2 T) streaming) and is computed once on the
host in float64; the device kernel is a pure bandwidth-bound stream:
out_shard = blockdiag(A,A) @ x_shard.

Per core the shard is pre-stacked on host as (128, 32768): rows 0:64 hold
the first 32768 columns, rows 64:128 the next 32768.  The stationary
matrix is the block-diagonal [[A^T, 0], [0, A^T]] (128x128), which doubles
PE utilization (K=128, M=128 instead of 64).

Bandwidth tricks (the target regime is the HBM ridge, ~430 GB/s/core
duplex; measured exec ~54-62us vs the 124us f32 single-queue baseline):
  * x and out travel as bfloat16 (f32 PSUM accumulation).  Halves HBM
    traffic; measured end-to-end rel err ~2.9e-3 vs the f32 reference.
  * Input DMAs issue from the SP (sync) HWDGE queue, output DMAs from the
    Activation (scalar) HWDGE queue.  One shared queue serializes loads
    behind stores that wait on compute (head-of-line blocking was the
    dominant stall in the 124us baseline).
  * Each chunk is its own contiguous (128 x cols) DRAM tensor.  The DGE
    emits one descriptor per partition either way, but descriptors whose
    DRAM side is contiguous cost ~280ns vs ~600ns for 64KiB-strided rows
    -- this doubled effective write bandwidth.
  * The whole input shard is prefetched from t=0 (xin pools hold all
    8 MiB), so loads never wait on compute; PSUM->SBUF cast-copies
    alternate DVE/Activation so neither engine paces the PE; 8 PSUM
    banks decouple matmul from copy jitter.
"""

import os
import sys

import numpy as np

for _p in ("/opt/trn_rl_repo", "/root/.axon_site/_ro/trn_rl_repo"):
    if _p not in sys.path and os.path.isdir(_p):
        sys.path.append(_p)

import ml_dtypes

import concourse.bass as bass
import concourse.mybir as mybir
from concourse import bacc
from concourse.bass_utils import run_bass_kernel_spmd

from concourse.tile import TileContext

F32 = mybir.dt.float32
BF16 = mybir.dt.bfloat16
BF16_NP = ml_dtypes.bfloat16

N = 64                   # state dim
N_CORES = 8
T_FULL = 524288          # columns of the reshaped X
T_CORE = T_FULL // N_CORES       # 65536 columns per core
T_HALF = T_CORE // 2             # 32768 -> free dim of the (128, .) shard

MM_COLS = 512            # matmul moving free dim (one PSUM bank, f32)

# Per-chunk DRAM tensors, each a contiguous (128 x cols) block.  The DGE
# still emits one descriptor per partition, but a *contiguous* DRAM side
# halves the per-descriptor cost of writes (~280ns vs ~600ns strided).
# Small leading input chunks start the PE early; 8192-col output chunks
# (16 KiB descriptors) halve the write descriptor count.
IN_CHUNKS = [4096] * 8
OUT_CHUNKS = [8192, 8192, 8192, 4096, 2048, 1024, 1024]


def build_kernel(t_half=T_HALF):
    assert sum(IN_CHUNKS) == t_half and sum(OUT_CHUNKS) == t_half
    nc = bacc.Bacc()

    x_exts = [
        nc.declare_dram_parameter(f"x{c}", [128, cols], BF16, isOutput=False)
        for c, cols in enumerate(IN_CHUNKS)
    ]
    at_ext = nc.declare_dram_parameter("AT128", [128, 128], BF16, isOutput=False)
    out_exts = [
        nc.declare_dram_parameter(f"out{c}", [128, cols], BF16, isOutput=True)
        for c, cols in enumerate(OUT_CHUNKS)
    ]

    with TileContext(nc) as tc:
        n_small_chunks = sum(1 for c in IN_CHUNKS if c < 4096)
        with (
            tc.tile_pool(name="const", bufs=1) as cpool,
            tc.tile_pool(name="xin_s", bufs=max(n_small_chunks, 1)) as xpool_s,
            tc.tile_pool(name="xin", bufs=len(IN_CHUNKS) - n_small_chunks) as xpool,
            tc.tile_pool(name="yout", bufs=4) as opool,
            tc.tile_pool(name="mm_ps", bufs=8, space="PSUM") as mps,
        ):
            # stationary blockdiag(A^T, A^T)
            at_sb = cpool.tile([128, 128], BF16)
            nc.sync.dma_start(out=at_sb[:], in_=at_ext[:, :])

            # prefetch the whole shard on the SP queue: loads only ever
            # wait on the queue itself, never on compute
            tile_src = {}  # 512-col tile index -> (xin tile, col offset)
            base = 0
            for c, cols in enumerate(IN_CHUNKS):
                pool = xpool_s if cols < 4096 else xpool
                xin = pool.tile([128, cols], BF16, tag="xin")
                nc.sync.dma_start(out=xin[:], in_=x_exts[c][:, :])
                for j in range(cols // MM_COLS):
                    tile_src[base // MM_COLS + j] = (xin, j * MM_COLS)
                base += cols

            # stream: matmul 512-col tiles into PSUM, cast-copy to bf16
            # SBUF (alternating DVE/Activation so neither paces the PE),
            # store each chunk from the Activation queue.  Tail chunks are
            # produced after the input stream has drained, so alternate
            # them onto the (now idle) SP queue to double tail write rate.
            tail_eng = {4: nc.sync, 6: nc.sync}
            obase = 0
            for c, cols in enumerate(OUT_CHUNKS):
                yout = opool.tile([128, cols], BF16, tag="yout", name="yout")
                for j in range(cols // MM_COLS):
                    g = obase // MM_COLS + j
                    xin, xoff = tile_src[g]
                    ps = mps.tile([128, MM_COLS], F32, tag="mm")
                    nc.tensor.matmul(
                        ps[:],
                        lhsT=at_sb[:],
                        rhs=xin[:, xoff : xoff + MM_COLS],
                        start=True,
                        stop=True,
                    )
                    dst = yout[:, j * MM_COLS : (j + 1) * MM_COLS]
                    if g % 2 == 0:
                        nc.vector.tensor_copy(out=dst, in_=ps[:])
                    else:
                        nc.scalar.copy(out=dst, in_=ps[:])
                eng = tail_eng.get(c, nc.scalar)
                eng.dma_start(out=out_exts[c][:, :], in_=yout[:])
                obase += cols

    return nc


_NC_CACHE = {}
LAST_PROFILE = None


def _get_nc(t_half=T_HALF):
    if t_half not in _NC_CACHE:
        nc = build_kernel(t_half)
        nc.finalize()  # Bacc: reg alloc + event-semaphore wait splitting
        _NC_CACHE[t_half] = nc
    return _NC_CACHE[t_half]


def _ensure_ntff_hook():
    """The agent image's `antenv` lacks the `axon_hooks` shim that
    `trn_agent_boot` uses to register the NTFF profiling hook (boot
    degrades silently).  Provide the shim and register the hook so
    run_bass_kernel_spmd(trace=True) can capture neuron-profile data."""
    import types

    try:
        from antenv.axon_hooks import get_axon_ntff_profile_hook  # noqa: F401
        return True
    except ImportError:
        pass
    try:
        import antenv
        from trn_agent_boot.trn_boot import _ntff_profile_via_ctypes

        mod = types.ModuleType("antenv.axon_hooks")
        _store = {"h": None}
        mod.set_axon_ntff_profile_hook = lambda h: _store.__setitem__("h", h)
        mod.get_axon_ntff_profile_hook = lambda: _store["h"]
        sys.modules["antenv.axon_hooks"] = mod
        antenv.axon_hooks = mod
        hook = _ntff_profile_via_ctypes("/opt/axon/libaxon_pjrt.so")
        mod.set_axon_ntff_profile_hook(hook)
        return hook is not None
    except Exception as e:  # degrade to no-trace
        print(f"kernel.py: NTFF hook setup failed ({type(e).__name__}: {e})")
        return False


def kernel(x, L, R):
    global LAST_PROFILE
    x = np.ascontiguousarray(np.asarray(x, dtype=np.float32))
    L = np.asarray(L, dtype=np.float32)
    R = np.asarray(R, dtype=np.float32)
    assert x.shape == (T_FULL, N), x.shape

    # tiny operator, host float64: A = inv(2*(M11+M22+R-R^T)) @ M21
    M = L.astype(np.float64) @ L.T.astype(np.float64)
    M += 1e-8 * np.eye(2 * N)
    B = 2.0 * (M[:N, :N] + M[N:, N:] + R.astype(np.float64) - R.T.astype(np.float64))
    A = np.linalg.solve(B, M[:N, N:])
    at128 = np.zeros((128, 128), dtype=BF16_NP)
    at128[:N, :N] = A.T.astype(BF16_NP)
    at128[N:, N:] = at128[:N, :N]

    X = x.reshape(N, T_FULL).astype(BF16_NP)  # round-to-nearest-even
    in_maps = []
    for c in range(N_CORES):
        base = c * T_CORE
        m = {"AT128": at128}
        cb = base
        for k, cols in enumerate(IN_CHUNKS):
            blk = np.empty((128, cols), dtype=BF16_NP)
            blk[:N] = X[:, cb : cb + cols]
            blk[N:] = X[:, cb + T_HALF : cb + T_HALF + cols]
            m[f"x{k}"] = blk
            cb += cols
        in_maps.append(m)

    nc = _get_nc()
    trace = os.environ.get("KERNEL_TRACE", "0") == "1"
    if trace:
        trace = _ensure_ntff_hook()
    try:
        res = run_bass_kernel_spmd(
            nc, in_maps, core_ids=list(range(N_CORES)), trace=trace
        )
    except Exception:
        if not trace:
            raise
        print("kernel.py: traced run failed; retrying without trace")
        res = run_bass_kernel_spmd(
            nc, in_maps, core_ids=list(range(N_CORES)), trace=False
        )
    LAST_PROFILE = res

    Y = np.empty((N, T_FULL), dtype=np.float32)
    for c in range(N_CORES):
        cb = c * T_CORE
        for k, cols in enumerate(OUT_CHUNKS):
            o = np.asarray(res.results[c][f"out{k}"]).astype(np.float32)
            Y[:, cb : cb + cols] = o[:N]
            Y[:, cb + T_HALF : cb + T_HALF + cols] = o[N:]
            cb += cols
    return Y.reshape(T_FULL, N)


# revision 32
# speedup vs baseline: 1.0185x; 1.0014x over previous
"""Distributed Trainium2 kernel for the Koopman-operator problem.

Math (from the reference):
    X  = x.reshape(64, T)                 # T = 524288, pure row-major view
    M  = L @ L.T                          # 128x128;  M11, M21, M22 are 64x64 blocks
    B  = 2*(M11 + M22 + R - R.T)          # (eps*I is ~1e-8, negligible vs O(30) entries)
    A  = inv(B) @ M21
    out = (A @ X).reshape(-1, 64)

Distribution: column-shard X across 8 cores (65536 cols each) -- fully
data-parallel, zero collectives.  The tiny 64x64 operator A is parameter
preprocessing (O(n^3) vs O(n^2 T) streaming) and is computed once on the
host in float64; the device kernel is a pure bandwidth-bound stream:
out_shard = blockdiag(A,A) @ x_shard.

Per core the shard is pre-stacked on host as (128, 32768): rows 0:64 hold
the first 32768 columns, rows 64:128 the next 32768.  The stationary
matrix is the block-diagonal [[A^T, 0], [0, A^T]] (128x128), which doubles
PE utilization (K=128, M=128 instead of 64).

Bandwidth tricks (the target regime is the HBM ridge, ~430 GB/s/core
duplex; measured exec ~54-62us vs the 124us f32 single-queue baseline):
  * x and out travel as bfloat16 (f32 PSUM accumulation).  Halves HBM
    traffic; measured end-to-end rel err ~2.9e-3 vs the f32 reference.
  * Input DMAs issue from the SP (sync) HWDGE queue, output DMAs from the
    Activation (scalar) HWDGE queue.  One shared queue serializes loads
    behind stores that wait on compute (head-of-line blocking was the
    dominant stall in the 124us baseline).
  * Each chunk is its own contiguous (128 x cols) DRAM tensor.  The DGE
    emits one descriptor per partition either way, but descriptors whose
    DRAM side is contiguous cost ~280ns vs ~600ns for 64KiB-strided rows
    -- this doubled effective write bandwidth.
  * The whole input shard is prefetched from t=0 (xin pools hold all
    8 MiB), so loads never wait on compute; PSUM->SBUF cast-copies
    alternate DVE/Activation so neither engine paces the PE; 8 PSUM
    banks decouple matmul from copy jitter.
"""

import os
import sys

import numpy as np

for _p in ("/opt/trn_rl_repo", "/root/.axon_site/_ro/trn_rl_repo"):
    if _p not in sys.path and os.path.isdir(_p):
        sys.path.append(_p)

import ml_dtypes

import concourse.bass as bass
import concourse.mybir as mybir
from concourse import bacc
from concourse.bass_utils import run_bass_kernel_spmd

from concourse.tile import TileContext

F32 = mybir.dt.float32
BF16 = mybir.dt.bfloat16
BF16_NP = ml_dtypes.bfloat16

N = 64                   # state dim
N_CORES = 8
T_FULL = 524288          # columns of the reshaped X
T_CORE = T_FULL // N_CORES       # 65536 columns per core
T_HALF = T_CORE // 2             # 32768 -> free dim of the (128, .) shard

MM_COLS = 512            # matmul moving free dim (one PSUM bank, f32)

# Per-chunk DRAM tensors, each a contiguous (128 x cols) block.  The DGE
# still emits one descriptor per partition, but a *contiguous* DRAM side
# halves the per-descriptor cost of writes (~280ns vs ~600ns strided).
# Small leading input chunks start the PE early; 8192-col output chunks
# (16 KiB descriptors) halve the write descriptor count.
IN_CHUNKS = [4096] * 8
OUT_CHUNKS = [4096, 8192, 8192, 8192, 2048, 1024, 1024]


def build_kernel(t_half=T_HALF):
    assert sum(IN_CHUNKS) == t_half and sum(OUT_CHUNKS) == t_half
    nc = bacc.Bacc()

    x_exts = [
        nc.declare_dram_parameter(f"x{c}", [128, cols], BF16, isOutput=False)
        for c, cols in enumerate(IN_CHUNKS)
    ]
    at_ext = nc.declare_dram_parameter("AT128", [128, 128], BF16, isOutput=False)
    out_exts = [
        nc.declare_dram_parameter(f"out{c}", [128, cols], BF16, isOutput=True)
        for c, cols in enumerate(OUT_CHUNKS)
    ]

    with TileContext(nc) as tc:
        n_small_chunks = sum(1 for c in IN_CHUNKS if c < 4096)
        with (
            tc.tile_pool(name="const", bufs=1) as cpool,
            tc.tile_pool(name="xin_s", bufs=max(n_small_chunks, 1)) as xpool_s,
            tc.tile_pool(name="xin", bufs=len(IN_CHUNKS) - n_small_chunks) as xpool,
            tc.tile_pool(name="yout", bufs=4) as opool,
            tc.tile_pool(name="mm_ps", bufs=8, space="PSUM") as mps,
        ):
            # stationary blockdiag(A^T, A^T)
            at_sb = cpool.tile([128, 128], BF16)
            nc.sync.dma_start(out=at_sb[:], in_=at_ext[:, :])

            # prefetch the whole shard on the SP queue: loads only ever
            # wait on the queue itself, never on compute
            tile_src = {}  # 512-col tile index -> (xin tile, col offset)
            base = 0
            for c, cols in enumerate(IN_CHUNKS):
                pool = xpool_s if cols < 4096 else xpool
                xin = pool.tile([128, cols], BF16, tag="xin")
                nc.sync.dma_start(out=xin[:], in_=x_exts[c][:, :])
                for j in range(cols // MM_COLS):
                    tile_src[base // MM_COLS + j] = (xin, j * MM_COLS)
                base += cols

            # stream: matmul 512-col tiles into PSUM, cast-copy to bf16
            # SBUF (alternating DVE/Activation so neither paces the PE),
            # store each chunk from the Activation queue.  Tail chunks are
            # produced after the input stream has drained, so alternate
            # them onto the (now idle) SP queue to double tail write rate.
            tail_eng = {4: nc.sync, 6: nc.sync}
            obase = 0
            for c, cols in enumerate(OUT_CHUNKS):
                yout = opool.tile([128, cols], BF16, tag="yout", name="yout")
                for j in range(cols // MM_COLS):
                    g = obase // MM_COLS + j
                    xin, xoff = tile_src[g]
                    ps = mps.tile([128, MM_COLS], F32, tag="mm")
                    nc.tensor.matmul(
                        ps[:],
                        lhsT=at_sb[:],
                        rhs=xin[:, xoff : xoff + MM_COLS],
                        start=True,
                        stop=True,
                    )
                    dst = yout[:, j * MM_COLS : (j + 1) * MM_COLS]
                    if g % 2 == 0:
                        nc.vector.tensor_copy(out=dst, in_=ps[:])
                    else:
                        nc.scalar.copy(out=dst, in_=ps[:])
                eng = tail_eng.get(c, nc.scalar)
                eng.dma_start(out=out_exts[c][:, :], in_=yout[:])
                obase += cols

    return nc


_NC_CACHE = {}
LAST_PROFILE = None


def _get_nc(t_half=T_HALF):
    if t_half not in _NC_CACHE:
        nc = build_kernel(t_half)
        nc.finalize()  # Bacc: reg alloc + event-semaphore wait splitting
        _NC_CACHE[t_half] = nc
    return _NC_CACHE[t_half]


def _ensure_ntff_hook():
    """The agent image's `antenv` lacks the `axon_hooks` shim that
    `trn_agent_boot` uses to register the NTFF profiling hook (boot
    degrades silently).  Provide the shim and register the hook so
    run_bass_kernel_spmd(trace=True) can capture neuron-profile data."""
    import types

    try:
        from antenv.axon_hooks import get_axon_ntff_profile_hook  # noqa: F401
        return True
    except ImportError:
        pass
    try:
        import antenv
        from trn_agent_boot.trn_boot import _ntff_profile_via_ctypes

        mod = types.ModuleType("antenv.axon_hooks")
        _store = {"h": None}
        mod.set_axon_ntff_profile_hook = lambda h: _store.__setitem__("h", h)
        mod.get_axon_ntff_profile_hook = lambda: _store["h"]
        sys.modules["antenv.axon_hooks"] = mod
        antenv.axon_hooks = mod
        hook = _ntff_profile_via_ctypes("/opt/axon/libaxon_pjrt.so")
        mod.set_axon_ntff_profile_hook(hook)
        return hook is not None
    except Exception as e:  # degrade to no-trace
        print(f"kernel.py: NTFF hook setup failed ({type(e).__name__}: {e})")
        return False


def kernel(x, L, R):
    global LAST_PROFILE
    x = np.ascontiguousarray(np.asarray(x, dtype=np.float32))
    L = np.asarray(L, dtype=np.float32)
    R = np.asarray(R, dtype=np.float32)
    assert x.shape == (T_FULL, N), x.shape

    # tiny operator, host float64: A = inv(2*(M11+M22+R-R^T)) @ M21
    M = L.astype(np.float64) @ L.T.astype(np.float64)
    M += 1e-8 * np.eye(2 * N)
    B = 2.0 * (M[:N, :N] + M[N:, N:] + R.astype(np.float64) - R.T.astype(np.float64))
    A = np.linalg.solve(B, M[:N, N:])
    at128 = np.zeros((128, 128), dtype=BF16_NP)
    at128[:N, :N] = A.T.astype(BF16_NP)
    at128[N:, N:] = at128[:N, :N]

    X = x.reshape(N, T_FULL).astype(BF16_NP)  # round-to-nearest-even
    in_maps = []
    for c in range(N_CORES):
        base = c * T_CORE
        m = {"AT128": at128}
        cb = base
        for k, cols in enumerate(IN_CHUNKS):
            blk = np.empty((128, cols), dtype=BF16_NP)
            blk[:N] = X[:, cb : cb + cols]
            blk[N:] = X[:, cb + T_HALF : cb + T_HALF + cols]
            m[f"x{k}"] = blk
            cb += cols
        in_maps.append(m)

    nc = _get_nc()
    trace = os.environ.get("KERNEL_TRACE", "0") == "1"
    if trace:
        trace = _ensure_ntff_hook()
    try:
        res = run_bass_kernel_spmd(
            nc, in_maps, core_ids=list(range(N_CORES)), trace=trace
        )
    except Exception:
        if not trace:
            raise
        print("kernel.py: traced run failed; retrying without trace")
        res = run_bass_kernel_spmd(
            nc, in_maps, core_ids=list(range(N_CORES)), trace=False
        )
    LAST_PROFILE = res

    Y = np.empty((N, T_FULL), dtype=np.float32)
    for c in range(N_CORES):
        cb = c * T_CORE
        for k, cols in enumerate(OUT_CHUNKS):
            o = np.asarray(res.results[c][f"out{k}"]).astype(np.float32)
            Y[:, cb : cb + cols] = o[:N]
            Y[:, cb + T_HALF : cb + T_HALF + cols] = o[N:]
            cb += cols
    return Y.reshape(T_FULL, N)


# revision 36
# speedup vs baseline: 1.0269x; 1.0083x over previous
"""Distributed Trainium2 kernel for the Koopman-operator problem.

Math (from the reference):
    X  = x.reshape(64, T)                 # T = 524288, pure row-major view
    M  = L @ L.T                          # 128x128;  M11, M21, M22 are 64x64 blocks
    B  = 2*(M11 + M22 + R - R.T)          # (eps*I is ~1e-8, negligible vs O(30) entries)
    A  = inv(B) @ M21
    out = (A @ X).reshape(-1, 64)

Distribution: column-shard X across 8 cores (65536 cols each) -- fully
data-parallel, zero collectives.  The tiny 64x64 operator A is parameter
preprocessing (O(n^3) vs O(n^2 T) streaming) and is computed once on the
host in float64; the device kernel is a pure bandwidth-bound stream:
out_shard = blockdiag(A,A) @ x_shard.

Per core the shard is pre-stacked on host as (128, 32768): rows 0:64 hold
the first 32768 columns, rows 64:128 the next 32768.  The stationary
matrix is the block-diagonal [[A^T, 0], [0, A^T]] (128x128), which doubles
PE utilization (K=128, M=128 instead of 64).

Bandwidth tricks (the target regime is the HBM ridge, ~430-450 GB/s/core
duplex; measured exec ~53-57us typical vs the 124us f32 single-queue
baseline):
  * x and out travel as bfloat16 (f32 PSUM accumulation).  Halves HBM
    traffic; measured end-to-end rel err ~2.9e-3 vs the f32 reference
    (fp8 input measures 2.7e-2 -- over the 2e-2 gate -- so bf16 is the
    byte floor).
  * Input DMAs issue from the SP (sync) HWDGE queue, output DMAs from the
    Activation (scalar) HWDGE queue.  One shared queue serializes loads
    behind stores that wait on compute (head-of-line blocking was the
    dominant stall in the 124us baseline).
  * Each chunk is its own contiguous (128 x cols) DRAM tensor.  The DGE
    emits one descriptor per partition either way, but descriptors whose
    DRAM side is contiguous cost ~280ns vs ~600ns for 64KiB-strided rows
    -- this doubled effective write bandwidth.
  * The whole input shard is prefetched from t=0 (xin pool holds all
    8 MiB), so loads never wait on compute; PSUM->SBUF cast-copies
    alternate DVE/Activation so neither engine paces the PE; 8 PSUM
    banks decouple matmul from copy jitter.
  * Output chunks taper (4096 head so stores start 8 tiles in, 8192
    middle, 2048/1024 tail): any chunk costs >=128 descriptors, so the
    post-compute drain is minimized by small final chunks, and alternate
    tail chunks ride the (by then idle) SP queue so both HWDGE queues
    drain the tail in parallel.
"""

import os
import sys

import numpy as np

for _p in ("/opt/trn_rl_repo", "/root/.axon_site/_ro/trn_rl_repo"):
    if _p not in sys.path and os.path.isdir(_p):
        sys.path.append(_p)

import ml_dtypes

import concourse.bass as bass
import concourse.mybir as mybir
from concourse import bacc
from concourse.bass_utils import run_bass_kernel_spmd

from concourse.tile import TileContext

F32 = mybir.dt.float32
BF16 = mybir.dt.bfloat16
BF16_NP = ml_dtypes.bfloat16

N = 64                   # state dim
N_CORES = 8
T_FULL = 524288          # columns of the reshaped X
T_CORE = T_FULL // N_CORES       # 65536 columns per core
T_HALF = T_CORE // 2             # 32768 -> free dim of the (128, .) shard

MM_COLS = 512            # matmul moving free dim (one PSUM bank, f32)

# Per-chunk DRAM tensors, each a contiguous (128 x cols) block.  The DGE
# still emits one descriptor per partition, but a *contiguous* DRAM side
# halves the per-descriptor cost of writes (~280ns vs ~600ns strided).
# Small leading input chunks start the PE early; 8192-col output chunks
# (16 KiB descriptors) halve the write descriptor count.
IN_CHUNKS = [4096] * 8
OUT_CHUNKS = [4096, 8192, 8192, 8192, 2048, 1024, 1024]


def build_kernel(t_half=T_HALF):
    assert sum(IN_CHUNKS) == t_half and sum(OUT_CHUNKS) == t_half
    nc = bacc.Bacc()

    x_exts = [
        nc.declare_dram_parameter(f"x{c}", [128, cols], BF16, isOutput=False)
        for c, cols in enumerate(IN_CHUNKS)
    ]
    at_ext = nc.declare_dram_parameter("AT128", [128, 128], BF16, isOutput=False)
    out_exts = [
        nc.declare_dram_parameter(f"out{c}", [128, cols], BF16, isOutput=True)
        for c, cols in enumerate(OUT_CHUNKS)
    ]

    with TileContext(nc) as tc:
        with (
            tc.tile_pool(name="const", bufs=1) as cpool,
            tc.tile_pool(name="xin", bufs=len(IN_CHUNKS)) as xpool,
            tc.tile_pool(name="yout", bufs=4) as opool,
            tc.tile_pool(name="mm_ps", bufs=8, space="PSUM") as mps,
        ):
            # stationary blockdiag(A^T, A^T)
            at_sb = cpool.tile([128, 128], BF16)
            nc.sync.dma_start(out=at_sb[:], in_=at_ext[:, :])

            # prefetch the whole shard on the SP queue: loads only ever
            # wait on the queue itself, never on compute
            tile_src = {}  # 512-col tile index -> (xin tile, col offset)
            base = 0
            for c, cols in enumerate(IN_CHUNKS):
                xin = xpool.tile([128, cols], BF16, tag="xin")
                nc.sync.dma_start(out=xin[:], in_=x_exts[c][:, :])
                for j in range(cols // MM_COLS):
                    tile_src[base // MM_COLS + j] = (xin, j * MM_COLS)
                base += cols

            # stream: matmul 512-col tiles into PSUM, cast-copy to bf16
            # SBUF (alternating DVE/Activation so neither paces the PE),
            # store each chunk from the Activation queue.  Tail chunks are
            # produced after the input stream has drained, so alternate
            # them onto the (now idle) SP queue to double tail write rate.
            tail_eng = {4: nc.sync, 6: nc.sync}
            obase = 0
            for c, cols in enumerate(OUT_CHUNKS):
                yout = opool.tile([128, cols], BF16, tag="yout", name="yout")
                for j in range(cols // MM_COLS):
                    g = obase // MM_COLS + j
                    xin, xoff = tile_src[g]
                    ps = mps.tile([128, MM_COLS], F32, tag="mm")
                    nc.tensor.matmul(
                        ps[:],
                        lhsT=at_sb[:],
                        rhs=xin[:, xoff : xoff + MM_COLS],
                        start=True,
                        stop=True,
                    )
                    dst = yout[:, j * MM_COLS : (j + 1) * MM_COLS]
                    if g % 2 == 0:
                        nc.vector.tensor_copy(out=dst, in_=ps[:])
                    else:
                        nc.scalar.copy(out=dst, in_=ps[:])
                eng = tail_eng.get(c, nc.scalar)
                eng.dma_start(out=out_exts[c][:, :], in_=yout[:])
                obase += cols

    return nc


_NC_CACHE = {}
LAST_PROFILE = None


def _get_nc(t_half=T_HALF):
    if t_half not in _NC_CACHE:
        nc = build_kernel(t_half)
        nc.finalize()  # Bacc: reg alloc + event-semaphore wait splitting
        _NC_CACHE[t_half] = nc
    return _NC_CACHE[t_half]


def _ensure_ntff_hook():
    """The agent image's `antenv` lacks the `axon_hooks` shim that
    `trn_agent_boot` uses to register the NTFF profiling hook (boot
    degrades silently).  Provide the shim and register the hook so
    run_bass_kernel_spmd(trace=True) can capture neuron-profile data."""
    import types

    try:
        from antenv.axon_hooks import get_axon_ntff_profile_hook  # noqa: F401
        return True
    except ImportError:
        pass
    try:
        import antenv
        from trn_agent_boot.trn_boot import _ntff_profile_via_ctypes

        mod = types.ModuleType("antenv.axon_hooks")
        _store = {"h": None}
        mod.set_axon_ntff_profile_hook = lambda h: _store.__setitem__("h", h)
        mod.get_axon_ntff_profile_hook = lambda: _store["h"]
        sys.modules["antenv.axon_hooks"] = mod
        antenv.axon_hooks = mod
        hook = _ntff_profile_via_ctypes("/opt/axon/libaxon_pjrt.so")
        mod.set_axon_ntff_profile_hook(hook)
        return hook is not None
    except Exception as e:  # degrade to no-trace
        print(f"kernel.py: NTFF hook setup failed ({type(e).__name__}: {e})")
        return False


def kernel(x, L, R):
    global LAST_PROFILE
    x = np.ascontiguousarray(np.asarray(x, dtype=np.float32))
    L = np.asarray(L, dtype=np.float32)
    R = np.asarray(R, dtype=np.float32)
    assert x.shape == (T_FULL, N), x.shape

    # tiny operator, host float64: A = inv(2*(M11+M22+R-R^T)) @ M21
    M = L.astype(np.float64) @ L.T.astype(np.float64)
    M += 1e-8 * np.eye(2 * N)
    B = 2.0 * (M[:N, :N] + M[N:, N:] + R.astype(np.float64) - R.T.astype(np.float64))
    A = np.linalg.solve(B, M[:N, N:])
    at128 = np.zeros((128, 128), dtype=BF16_NP)
    at128[:N, :N] = A.T.astype(BF16_NP)
    at128[N:, N:] = at128[:N, :N]

    X = x.reshape(N, T_FULL).astype(BF16_NP)  # round-to-nearest-even
    in_maps = []
    for c in range(N_CORES):
        base = c * T_CORE
        m = {"AT128": at128}
        cb = base
        for k, cols in enumerate(IN_CHUNKS):
            blk = np.empty((128, cols), dtype=BF16_NP)
            blk[:N] = X[:, cb : cb + cols]
            blk[N:] = X[:, cb + T_HALF : cb + T_HALF + cols]
            m[f"x{k}"] = blk
            cb += cols
        in_maps.append(m)

    nc = _get_nc()
    trace = os.environ.get("KERNEL_TRACE", "0") == "1"
    if trace:
        trace = _ensure_ntff_hook()
    try:
        res = run_bass_kernel_spmd(
            nc, in_maps, core_ids=list(range(N_CORES)), trace=trace
        )
    except Exception:
        if not trace:
            raise
        print("kernel.py: traced run failed; retrying without trace")
        res = run_bass_kernel_spmd(
            nc, in_maps, core_ids=list(range(N_CORES)), trace=False
        )
    LAST_PROFILE = res

    Y = np.empty((N, T_FULL), dtype=np.float32)
    for c in range(N_CORES):
        cb = c * T_CORE
        for k, cols in enumerate(OUT_CHUNKS):
            o = np.asarray(res.results[c][f"out{k}"]).astype(np.float32)
            Y[:, cb : cb + cols] = o[:N]
            Y[:, cb + T_HALF : cb + T_HALF + cols] = o[N:]
            cb += cols
    return Y.reshape(T_FULL, N)
